# revision 28
# baseline (speedup 1.0000x reference)
"""Causal single-head attention (B=16, S=2048, D=1024, HD=64) on 8 TRN2 cores.

Data-parallel: 2 batches per core, bf16 compute with fp32 PSUM accumulation.

Host prep: x is converted to bf16 and pre-transposed to X^T layout
[128, b, d_block, s] so the kernel DMAs it straight into SBUF as the
projection moving operand (4KB contiguous lines, no on-device transpose).

Per seq-tile unit (b, st) of 512 rows:
  - pass1: stationary [Wq|Wk] over X^T -> PSUM [Q^T; K^T], bias-add copy
    to SBUF bf16 (vector),
  - pass2: stationary Wv -> V^T, then 4 PE transposes -> V natural with a
    ones column appended (the ones column makes the O matmul emit softmax
    denominators for free),
  - attention: S^T = K^T_blk^T @ Q^T per kseq-block with causal block
    skipping and diagonal narrowing; blocks processed in PAIRS sharing one
    2-bank PSUM group so one ScalarE exp instruction covers 1024 columns;
    diagonal masking on gpsimd (bf16 SBUF-only), O^T accumulated with
    stationary [V | ones],
  - finalize: O^T -> bf16, PE-transpose back to natural, multiply by
    1/denominator (vector), DMA out fp32.

Emission is software-pipelined as in the fp32 baseline: future units'
projection work is interleaved between attention groups to keep the PE
dense while ScalarE streams exps.
"""
import numpy as np
import ml_dtypes

import concourse.bacc as bacc
import concourse.mybir as mybir
import concourse.tile as tile
from concourse import bass_utils

B, S, D, HD = 16, 2048, 1024, 64
N_CORES = 8
BPC = B // N_CORES          # batches per core
ST = 512                    # seq tile (qtile) size
NST = S // ST               # 4 seq tiles per batch
NDB = D // 128              # 8 d-blocks
NKB = S // 128              # 16 kseq blocks per batch

f32 = mybir.dt.float32
bf16 = mybir.dt.bfloat16
bf16_np = ml_dtypes.bfloat16

# consts_bf16 layout (free dim): ident 128 | mask 128 | wqk 8*128 | wv 8*64 | ones 16
CB = 128 + 128 + NDB * 128 + NDB * 64 + 16

_cache = {}


def _build():
    nc = bacc.Bacc("TRN2", target_bir_lowering=False, debug=False,
                   num_devices=N_CORES)

    xT = nc.dram_tensor("xT", [128, BPC, NST, NDB, ST], bf16,
                        kind="ExternalInput")
    cb = nc.dram_tensor("cb", [128, CB], bf16, kind="ExternalInput")
    cf = nc.dram_tensor("cf", [128, 2], f32, kind="ExternalInput")
    out = nc.dram_tensor("out", [BPC, S, HD], f32, kind="ExternalOutput")

    with tile.TileContext(nc) as tc:
        with (
            tc.tile_pool(name="consts", bufs=1) as consts,
            tc.tile_pool(name="xsb", bufs=1) as xsb,
            tc.tile_pool(name="perbatch", bufs=1) as perbatch,
            tc.tile_pool(name="work", bufs=3) as work,
            tc.tile_pool(name="pp", bufs=1, space="PSUM") as pp,
            tc.tile_pool(name="ps_s", bufs=2, space="PSUM") as ps_s,
            tc.tile_pool(name="ps_o", bufs=1, space="PSUM") as ps_o,
            tc.tile_pool(name="ps_t", bufs=1, space="PSUM") as ps_t,
        ):
            cbt = consts.tile([128, CB], bf16)
            # weights-first split: readers of each region wait only on the
            # DMA that writes it, so projections start before mask/wqv land
            nc.sync.dma_start(out=cbt[:, 0:512], in_=cb.ap()[:, 0:512])
            cft = consts.tile([128, 2], f32)
            nc.scalar.dma_start(out=cft, in_=cf.ap())
            nc.scalar.dma_start(out=cbt[:, 512:], in_=cb.ap()[:, 512:])
            wk = cbt[:, 0:NDB * 64].rearrange(
                "p (db m) -> p db m", db=NDB)
            ident = cbt[:, 512:640]
            wqv = cbt[:, 640:640 + NDB * 128].rearrange(
                "p (db m) -> p db m", db=NDB)
            mask = cbt[:, 1664:1792]
            ones_c = cbt[:, 1792:1792 + NKB]
            bias_k = cft[:, 0:1]
            bias_qv = cft[:, 1:2]

            xt_sbs, kq_sbs, v_sbs = [], [], []
            xt_loaded = set()

            def load_xt_chunk(i):
                if i in xt_loaded or i >= BPC * NST:
                    return
                xt_loaded.add(i)
                b, st = divmod(i, NST)
                if i == 0:
                    # db-single/pair chunks: pass1 accumulates in db order,
                    # so unit 0 starts after 128KB instead of 1MB
                    for lo, hi in ((0, 1), (1, 2), (2, 4), (4, 6), (6, 8)):
                        nc.sync.dma_start(
                            out=xt_sbs[0][:, 0, lo:hi, :],
                            in_=xT.ap()[:, 0, 0, lo:hi, :])
                elif i == 1:
                    for h in range(2):
                        nc.scalar.dma_start(
                            out=xt_sbs[0][:, 1, 4 * h:4 * h + 4, :],
                            in_=xT.ap()[:, 0, 1, 4 * h:4 * h + 4, :])
                else:
                    eng = nc.sync if i % 2 == 0 else nc.scalar
                    eng.dma_start(out=xt_sbs[b][:, st],
                                  in_=xT.ap()[:, b, st])

            for b in range(BPC):
                xt_sb = xsb.tile([128, NST, NDB, ST], bf16, name=f"xt{b}")
                xt_sbs.append(xt_sb)
                if b == 0:
                    # PE warmup while X^T streams in: harmless matmuls on the
                    # consts tile ramp the tensor engine p-state early
                    wu = ps_s.tile([128, 2, ST], f32, tag="s", name="sg")
                    for j in (0, 1):
                        nc.tensor.matmul(wu[:, j, :], cbt[:, 0:128],
                                         cbt[:, 0:ST], start=True, stop=True)
                    # preload ScalarE's exp table during the DMA window
                    tbl = work.tile([128, 1], bf16, tag="rc", name="tblwarm")
                    nc.scalar.activation(
                        out=tbl, in_=wu[:, 0, 0:1],
                        func=mybir.ActivationFunctionType.Exp)
                # K^T per seq tile, partitions 0:64
                kq_sb = perbatch.tile([64, NST, ST], bf16, name=f"kq{b}")
                # V natural with ones column; rows padded to 128 cols so
                # crossbar-transpose writes land 256B-aligned
                v_sb = perbatch.tile([128, NKB, 128], bf16, name=f"v{b}")
                nc.gpsimd.tensor_copy(
                    out=v_sb[:, :, HD:HD + 1],
                    in_=ones_c.rearrange("p (t o) -> p t o", o=1))
                kq_sbs.append(kq_sb)
                v_sbs.append(v_sb)

            for i in range(3):
                load_xt_chunk(i)

            units = [(b, st) for b in range(BPC) for st in range(NST)]

            def filler_gen(b, st):
                """Emits unit (b, st)'s projections and V-natural prep
                incrementally (one yield per matmul). Returns the unit's
                transient Q^T/V^T tile via the final yield."""
                load_xt_chunk(b * NST + st + 2)
                xt_sb = xt_sbs[b]
                mov = xt_sb[:, st]
                p1 = pp.tile([64, ST], f32, tag="p1", name="p1")
                for db in range(NDB):
                    nc.tensor.matmul(p1, wk[:, db, :], mov[:, db, :],
                                     start=(db == 0), stop=(db == NDB - 1))
                    yield None
                nc.vector.tensor_scalar_add(out=kq_sbs[b][:, st, :], in0=p1,
                                            scalar1=bias_k[0:64, :])
                p2 = pp.tile([128, ST], f32, tag="p2", name="p2")
                for db in range(NDB):
                    nc.tensor.matmul(p2, wqv[:, db, :], mov[:, db, :],
                                     start=(db == 0), stop=(db == NDB - 1))
                    yield None
                # rows 0:64 = Q^T (S moving operand), 64:128 = V^T
                qv_sb = work.tile([128, ST], bf16, tag="qv", bufs=4,
                                  name="qv_sb")
                nc.vector.tensor_scalar_add(out=qv_sb, in0=p2,
                                            scalar1=bias_qv)
                vn_ps = ps_t.tile([128, 4, HD], bf16, tag="t", name="vn_ps")
                for c in range(4):
                    nc.tensor.transpose(
                        vn_ps[:, c, :],
                        qv_sb[64:128, 128 * c:128 * (c + 1)],
                        ident[64:128, 64:128])
                    yield None
                nc.vector.tensor_copy(
                    out=v_sbs[b][:, 4 * st:4 * st + 4, 0:HD], in_=vn_ps)
                yield qv_sb

            def emit_finalize(b, st, o_ps, last=False):
                o_qt = work.tile([128, NST, HD], f32, tag="oq", name="o_qt")
                recip = work.tile([128, NST], f32, tag="rc", name="recip")
                ot = work.tile([HD + 1, ST], bf16, tag="ot", name="ot")
                nc.vector.tensor_copy(out=ot, in_=o_ps)
                on_ps = ps_s.tile([128, NST, HD + 2], bf16, tag="s",
                                  name="on_ps")
                for c in range(NST):
                    nc.tensor.transpose(
                        on_ps[:, c, 0:HD + 1],
                        ot[:, 128 * c:128 * (c + 1)],
                        ident[0:HD + 1, 0:HD + 1])
                nc.vector.reciprocal(out=recip, in_=on_ps[:, :, HD:HD + 1])
                for c in range(NST):
                    nc.vector.tensor_scalar_mul(
                        out=o_qt[:, c, :], in0=on_ps[:, c, 0:HD],
                        scalar1=recip[:, c:c + 1])
                nc.sync.dma_start(
                    out=out.ap()[b, ST * st:ST * (st + 1), :]
                    .rearrange("(t p) d -> p t d", p=128),
                    in_=o_qt)

            def fill_chain():
                for i, (b, st) in enumerate(units):
                    g = filler_gen(b, st)
                    for r in g:
                        if r is not None:
                            yield ("unit", i, r)
                        else:
                            yield ("step", i, None)

            chain = fill_chain()
            qv_ready = {}
            exhausted = [False]

            def pull_until_unit(i):
                for kind, ui, r in chain:
                    if kind == "unit":
                        qv_ready[ui] = r
                        if ui >= i:
                            return

            def pull_steps(n):
                got = 0
                while got < n:
                    try:
                        kind, ui, r = next(chain)
                    except StopIteration:
                        exhausted[0] = True
                        return
                    if kind == "unit":
                        qv_ready[ui] = r
                    else:
                        got += 1

            deferred = [None]

            def flush_and_finalize(b, st, v_sb, o_box, pends, last=False):
                o_ps = get_o_ps(o_box)
                for n, (pw, poff, pkb, pe) in enumerate(pends):
                    nc.tensor.matmul(
                        o_ps[:, poff:poff + pw],
                        v_sb[:, pkb, 0:HD + 1],
                        pe[:, 0:pw],
                        start=(pkb == 0), stop=(n == len(pends) - 1))
                    pull_steps(1)
                emit_finalize(b, st, o_ps, last=last)

            def get_o_ps(o_box):
                if o_box[0] is None:
                    o_box[0] = ps_o.tile([HD + 1, ST], f32, name="o_ps")
                return o_box[0]

            # attention order: smallest unit last to shrink the exp-paced
            # tail (fills still emitted in natural unit-index order)
            att_order = [0, 1, 2, 3, 5, 6, 4, 7]
            for an, i in enumerate(att_order):
                b, st = units[i]
                if i not in qv_ready:
                    pull_until_unit(i)
                kq_sb, v_sb = kq_sbs[b], v_sbs[b]
                qT = qv_ready.pop(i)[0:64, :]
                o_box = [None]
                n_att = 4 * st + 4
                n_grp = n_att // 2
                pends = []  # (w, qoff, kb, e_ap)
                for g in range(n_grp):
                    sg = ps_s.tile([128, 2, ST], f32, tag="s", name="sg")
                    eg = work.tile([128, 2, ST], bf16, tag="e", bufs=4,
                                   name="eg")
                    widths = []
                    for j in (0, 1):
                        kb = 2 * g + j
                        jj = kb - 4 * st
                        if jj < 0:
                            w, qoff = ST, 0
                        else:
                            w, qoff = ST - 128 * jj, 128 * jj
                        nc.tensor.matmul(
                            sg[:, j, 0:w],
                            kq_sb[0:64, kb // 4,
                                  128 * (kb % 4):128 * (kb % 4) + 128],
                            qT[:, qoff:qoff + w],
                            start=True, stop=True)
                        widths.append((w, qoff, kb, jj))
                    if widths[0][0] == ST and widths[1][0] == ST:
                        nc.scalar.activation(
                            out=eg, in_=sg,
                            func=mybir.ActivationFunctionType.Exp,
                            scale=float(HD) ** -0.5)
                    else:
                        for j, (w, qoff, kb, jj) in enumerate(widths):
                            nc.scalar.activation(
                                out=eg[:, j, 0:w], in_=sg[:, j, 0:w],
                                func=mybir.ActivationFunctionType.Exp,
                                scale=float(HD) ** -0.5)
                    for j, (w, qoff, kb, jj) in enumerate(widths):
                        if jj >= 0:
                            nc.gpsimd.tensor_mul(out=eg[:, j, 0:128],
                                                 in0=eg[:, j, 0:128],
                                                 in1=mask)
                        pends.append((w, qoff, kb, eg[:, j, :]))
                    if g == 0 and deferred[0] is not None:
                        flush_and_finalize(*deferred[0])
                        deferred[0] = None
                    while len(pends) > 4:
                        pw, poff, pkb, pe = pends.pop(0)
                        nc.tensor.matmul(
                            get_o_ps(o_box)[:, poff:poff + pw],
                            v_sb[:, pkb, 0:HD + 1],
                            pe[:, 0:pw],
                            start=(pkb == 0), stop=False)
                    pull_steps({0: 8, 1: 5, 2: 3, 3: 2}[st])
                deferred[0] = (b, st, v_sb, o_box, pends)
            flush_and_finalize(*deferred[0], last=True)

    nc.compile()
    return nc


def _to_bf16(a):
    return np.asarray(a, dtype=np.float32).astype(bf16_np)


def _pack_consts(wq, wk, wv_, bq, bk, bv):
    cbt = np.zeros((128, CB), dtype=np.float32)
    # wk[p, db, :] = Wk[db*128+p, :]
    cbt[:, 0:NDB * 64] = (
        wk.reshape(NDB, 128, HD).transpose(1, 0, 2).reshape(128, NDB * 64))
    cbt[:, 512:640] = np.eye(128, dtype=np.float32)
    # wqv[p, db, 0:64] = Wq[db*128+p, :]; [64:128] = Wv
    wqv = np.concatenate(
        [wq.reshape(NDB, 128, HD).transpose(1, 0, 2),
         wv_.reshape(NDB, 128, HD).transpose(1, 0, 2)], axis=2)
    cbt[:, 640:640 + NDB * 128] = wqv.reshape(128, NDB * 128)
    # mask[k, q] = 1.0 where q >= k (keep)
    cbt[:, 1664:1792] = (np.arange(128)[None, :] >= np.arange(128)[:, None])
    cbt[:, 1792:1792 + NKB] = 1.0

    cft = np.zeros((128, 2), dtype=np.float32)
    cft[0:HD, 0] = bk
    cft[:, 1] = np.concatenate([bq, bv])
    return cbt.astype(bf16_np), np.ascontiguousarray(cft)


def kernel(x, Wq, bq, Wk, bk, Wv, bv):
    if "nc" not in _cache:
        _cache["nc"] = _build()
    nc = _cache["nc"]

    cbt, cft = _pack_consts(
        np.asarray(Wq, np.float32), np.asarray(Wk, np.float32),
        np.asarray(Wv, np.float32), np.asarray(bq, np.float32),
        np.asarray(bk, np.float32), np.asarray(bv, np.float32))

    # X^T layout per core: [128 p, b, db, s] with p the within-block d index
    xb = _to_bf16(x)  # [B, S, D]
    in_maps = []
    for c in range(N_CORES):
        xc = xb[c * BPC:(c + 1) * BPC]  # [BPC, S, D]
        # [128 p, b, st, db, s_local]
        xt = np.ascontiguousarray(
            xc.reshape(BPC, NST, ST, NDB, 128).transpose(4, 0, 1, 3, 2))
        in_maps.append({"xT": xt, "cb": cbt, "cf": cft})

    res = bass_utils.run_bass_kernel_spmd(nc, in_maps,
                                          core_ids=list(range(N_CORES)),
                                          **_cache.get("run_kwargs", {}))
    _cache["last_result"] = res
    return np.concatenate([res.results[c]["out"] for c in range(N_CORES)],
                          axis=0)


# revision 29
# speedup vs baseline: 1.0497x; 1.0497x over previous
"""Causal single-head attention (B=16, S=2048, D=1024, HD=64) on 8 TRN2 cores.

Data-parallel: 2 batches per core, bf16 compute with fp32 PSUM accumulation.

Host prep: x is converted to bf16 and pre-transposed to X^T layout
[128, b, d_block, s] so the kernel DMAs it straight into SBUF as the
projection moving operand (4KB contiguous lines, no on-device transpose).

Per seq-tile unit (b, st) of 512 rows:
  - pass1: stationary [Wq|Wk] over X^T -> PSUM [Q^T; K^T], bias-add copy
    to SBUF bf16 (vector),
  - pass2: stationary Wv -> V^T, then 4 PE transposes -> V natural with a
    ones column appended (the ones column makes the O matmul emit softmax
    denominators for free),
  - attention: S^T = K^T_blk^T @ Q^T per kseq-block with causal block
    skipping and diagonal narrowing; blocks processed in PAIRS sharing one
    2-bank PSUM group so one ScalarE exp instruction covers 1024 columns;
    diagonal masking on gpsimd (bf16 SBUF-only), O^T accumulated with
    stationary [V | ones],
  - finalize: O^T -> bf16, PE-transpose back to natural, multiply by
    1/denominator (vector), DMA out fp32.

Emission is software-pipelined as in the fp32 baseline: future units'
projection work is interleaved between attention groups to keep the PE
dense while ScalarE streams exps.
"""
import numpy as np
import ml_dtypes

import concourse.bacc as bacc
import concourse.mybir as mybir
import concourse.tile as tile
from concourse import bass_utils

B, S, D, HD = 16, 2048, 1024, 64
N_CORES = 8
BPC = B // N_CORES          # batches per core
ST = 512                    # seq tile (qtile) size
NST = S // ST               # 4 seq tiles per batch
NDB = D // 128              # 8 d-blocks
NKB = S // 128              # 16 kseq blocks per batch

f32 = mybir.dt.float32
bf16 = mybir.dt.bfloat16
bf16_np = ml_dtypes.bfloat16

# consts_bf16 layout (free dim): ident 128 | mask 128 | wqk 8*128 | wv 8*64 | ones 16
CB = 128 + 128 + NDB * 128 + NDB * 64 + 16

_cache = {}


def _build():
    nc = bacc.Bacc("TRN2", target_bir_lowering=False, debug=False,
                   num_devices=N_CORES)

    xT = nc.dram_tensor("xT", [128, BPC, NST, NDB, ST], bf16,
                        kind="ExternalInput")
    cb = nc.dram_tensor("cb", [128, CB], bf16, kind="ExternalInput")
    cf = nc.dram_tensor("cf", [128, 2], f32, kind="ExternalInput")
    out = nc.dram_tensor("out", [BPC, S, HD], f32, kind="ExternalOutput")

    with tile.TileContext(nc) as tc:
        with (
            tc.tile_pool(name="consts", bufs=1) as consts,
            tc.tile_pool(name="xsb", bufs=1) as xsb,
            tc.tile_pool(name="perbatch", bufs=1) as perbatch,
            tc.tile_pool(name="work", bufs=3) as work,
            tc.tile_pool(name="pp", bufs=1, space="PSUM") as pp,
            tc.tile_pool(name="ps_s", bufs=2, space="PSUM") as ps_s,
            tc.tile_pool(name="ps_o", bufs=1, space="PSUM") as ps_o,
            tc.tile_pool(name="ps_t", bufs=1, space="PSUM") as ps_t,
        ):
            cbt = consts.tile([128, CB], bf16)
            # weights-first split: readers of each region wait only on the
            # DMA that writes it, so projections start before mask/wqv land
            nc.sync.dma_start(out=cbt[:, 0:512], in_=cb.ap()[:, 0:512])
            cft = consts.tile([128, 2], f32)
            nc.scalar.dma_start(out=cft, in_=cf.ap())
            nc.scalar.dma_start(out=cbt[:, 512:], in_=cb.ap()[:, 512:])
            wk = cbt[:, 0:NDB * 64].rearrange(
                "p (db m) -> p db m", db=NDB)
            ident = cbt[:, 512:640]
            wqv = cbt[:, 640:640 + NDB * 128].rearrange(
                "p (db m) -> p db m", db=NDB)
            mask = cbt[:, 1664:1792]
            ones_c = cbt[:, 1792:1792 + NKB]
            bias_k = cft[:, 0:1]
            bias_qv = cft[:, 1:2]

            xt_sbs, kq_sbs, v_sbs = [], [], []
            xt_loaded = set()

            def load_xt_chunk(i):
                if i in xt_loaded or i >= BPC * NST:
                    return
                xt_loaded.add(i)
                b, st = divmod(i, NST)
                if i == 0:
                    # db-single/pair chunks: pass1 accumulates in db order,
                    # so unit 0 starts after 128KB instead of 1MB
                    for lo, hi in ((0, 1), (1, 2), (2, 4), (4, 6), (6, 8)):
                        nc.sync.dma_start(
                            out=xt_sbs[0][:, 0, lo:hi, :],
                            in_=xT.ap()[:, 0, 0, lo:hi, :])
                elif i == 1:
                    for h in range(2):
                        nc.scalar.dma_start(
                            out=xt_sbs[0][:, 1, 4 * h:4 * h + 4, :],
                            in_=xT.ap()[:, 0, 1, 4 * h:4 * h + 4, :])
                else:
                    eng = nc.sync if i % 2 == 0 else nc.scalar
                    eng.dma_start(out=xt_sbs[b][:, st],
                                  in_=xT.ap()[:, b, st])

            for b in range(BPC):
                xt_sb = xsb.tile([128, NST, NDB, ST], bf16, name=f"xt{b}")
                xt_sbs.append(xt_sb)
                if b == 0:
                    # PE warmup while X^T streams in: harmless matmuls on the
                    # consts tile ramp the tensor engine p-state early
                    wu = ps_s.tile([128, 2, ST], f32, tag="s", name="sg")
                    for j in (0, 1):
                        nc.tensor.matmul(wu[:, j, :], cbt[:, 0:128],
                                         cbt[:, 0:ST], start=True, stop=True)
                    # preload ScalarE's exp table during the DMA window
                    tbl = work.tile([128, 1], bf16, tag="rc", name="tblwarm")
                    nc.scalar.activation(
                        out=tbl, in_=wu[:, 0, 0:1],
                        func=mybir.ActivationFunctionType.Exp)
                # K^T per seq tile, partitions 0:64
                kq_sb = perbatch.tile([64, NST, ST], bf16, name=f"kq{b}")
                # V natural with ones column; rows padded to 128 cols so
                # crossbar-transpose writes land 256B-aligned
                v_sb = perbatch.tile([128, NKB, 128], bf16, name=f"v{b}")
                nc.gpsimd.tensor_copy(
                    out=v_sb[:, :, HD:HD + 1],
                    in_=ones_c.rearrange("p (t o) -> p t o", o=1))
                kq_sbs.append(kq_sb)
                v_sbs.append(v_sb)

            for i in range(BPC * NST):
                load_xt_chunk(i)

            units = [(b, st) for b in range(BPC) for st in range(NST)]

            def filler_gen(b, st):
                """Emits unit (b, st)'s projections and V-natural prep
                incrementally (one yield per matmul). Returns the unit's
                transient Q^T/V^T tile via the final yield."""
                load_xt_chunk(b * NST + st + 2)
                xt_sb = xt_sbs[b]
                mov = xt_sb[:, st]
                p1 = pp.tile([64, ST], f32, tag="p1", name="p1")
                for db in range(NDB):
                    nc.tensor.matmul(p1, wk[:, db, :], mov[:, db, :],
                                     start=(db == 0), stop=(db == NDB - 1))
                    yield None
                nc.vector.tensor_scalar_add(out=kq_sbs[b][:, st, :], in0=p1,
                                            scalar1=bias_k[0:64, :])
                p2 = pp.tile([128, ST], f32, tag="p2", name="p2")
                for db in range(NDB):
                    nc.tensor.matmul(p2, wqv[:, db, :], mov[:, db, :],
                                     start=(db == 0), stop=(db == NDB - 1))
                    yield None
                # rows 0:64 = Q^T (S moving operand), 64:128 = V^T
                qv_sb = work.tile([128, ST], bf16, tag="qv", bufs=4,
                                  name="qv_sb")
                nc.vector.tensor_scalar_add(out=qv_sb, in0=p2,
                                            scalar1=bias_qv)
                vn_ps = ps_t.tile([128, 4, HD], bf16, tag="t", name="vn_ps")
                for c in range(4):
                    nc.tensor.transpose(
                        vn_ps[:, c, :],
                        qv_sb[64:128, 128 * c:128 * (c + 1)],
                        ident[64:128, 64:128])
                    yield None
                nc.vector.tensor_copy(
                    out=v_sbs[b][:, 4 * st:4 * st + 4, 0:HD], in_=vn_ps)
                yield qv_sb

            def emit_finalize(b, st, o_ps, last=False):
                o_qt = work.tile([128, NST, HD], f32, tag="oq", name="o_qt")
                recip = work.tile([128, NST], f32, tag="rc", name="recip")
                ot = work.tile([HD + 1, ST], bf16, tag="ot", name="ot")
                nc.vector.tensor_copy(out=ot, in_=o_ps)
                on_ps = ps_t.tile([128, NST, HD + 2], bf16, tag="t",
                                  name="on_ps")
                for c in range(NST):
                    nc.tensor.transpose(
                        on_ps[:, c, 0:HD + 1],
                        ot[:, 128 * c:128 * (c + 1)],
                        ident[0:HD + 1, 0:HD + 1])
                nc.vector.reciprocal(out=recip, in_=on_ps[:, :, HD:HD + 1])
                for c in range(NST):
                    nc.vector.tensor_scalar_mul(
                        out=o_qt[:, c, :], in0=on_ps[:, c, 0:HD],
                        scalar1=recip[:, c:c + 1])
                nc.sync.dma_start(
                    out=out.ap()[b, ST * st:ST * (st + 1), :]
                    .rearrange("(t p) d -> p t d", p=128),
                    in_=o_qt)

            def fill_chain():
                for i, (b, st) in enumerate(units):
                    g = filler_gen(b, st)
                    for r in g:
                        if r is not None:
                            yield ("unit", i, r)
                        else:
                            yield ("step", i, None)

            chain = fill_chain()
            qv_ready = {}
            exhausted = [False]

            def pull_until_unit(i):
                for kind, ui, r in chain:
                    if kind == "unit":
                        qv_ready[ui] = r
                        if ui >= i:
                            return

            def pull_steps(n):
                got = 0
                while got < n:
                    try:
                        kind, ui, r = next(chain)
                    except StopIteration:
                        exhausted[0] = True
                        return
                    if kind == "unit":
                        qv_ready[ui] = r
                    else:
                        got += 1

            deferred = [None]

            def flush_and_finalize(b, st, v_sb, o_box, pends, last=False):
                o_ps = get_o_ps(o_box)
                for n, (pw, poff, pkb, pe) in enumerate(pends):
                    nc.tensor.matmul(
                        o_ps[:, poff:poff + pw],
                        v_sb[:, pkb, 0:HD + 1],
                        pe[:, 0:pw],
                        start=(pkb == 0), stop=(n == len(pends) - 1))
                    pull_steps(1)
                emit_finalize(b, st, o_ps, last=last)

            def get_o_ps(o_box):
                if o_box[0] is None:
                    o_box[0] = ps_o.tile([HD + 1, ST], f32, name="o_ps")
                return o_box[0]

            # attention order: smallest unit last to shrink the exp-paced
            # tail (fills still emitted in natural unit-index order)
            att_order = [0, 1, 2, 3, 5, 6, 4, 7]
            for an, i in enumerate(att_order):
                b, st = units[i]
                if i not in qv_ready:
                    pull_until_unit(i)
                kq_sb, v_sb = kq_sbs[b], v_sbs[b]
                qT = qv_ready.pop(i)[0:64, :]
                o_box = [None]
                n_att = 4 * st + 4
                n_grp = n_att // 2
                pends = []  # (w, qoff, kb, e_ap)
                for g in range(n_grp):
                    sg = ps_s.tile([128, 2, ST], f32, tag="s", name="sg")
                    eg = work.tile([128, 2, ST], bf16, tag="e", bufs=4,
                                   name="eg")
                    widths = []
                    for j in (0, 1):
                        kb = 2 * g + j
                        jj = kb - 4 * st
                        if jj < 0:
                            w, qoff = ST, 0
                        else:
                            w, qoff = ST - 128 * jj, 128 * jj
                        nc.tensor.matmul(
                            sg[:, j, 0:w],
                            kq_sb[0:64, kb // 4,
                                  128 * (kb % 4):128 * (kb % 4) + 128],
                            qT[:, qoff:qoff + w],
                            start=True, stop=True)
                        widths.append((w, qoff, kb, jj))
                    if widths[0][0] == ST and widths[1][0] == ST:
                        nc.scalar.activation(
                            out=eg, in_=sg,
                            func=mybir.ActivationFunctionType.Exp,
                            scale=float(HD) ** -0.5)
                    else:
                        for j, (w, qoff, kb, jj) in enumerate(widths):
                            nc.scalar.activation(
                                out=eg[:, j, 0:w], in_=sg[:, j, 0:w],
                                func=mybir.ActivationFunctionType.Exp,
                                scale=float(HD) ** -0.5)
                    for j, (w, qoff, kb, jj) in enumerate(widths):
                        if jj >= 0:
                            nc.gpsimd.tensor_mul(out=eg[:, j, 0:128],
                                                 in0=eg[:, j, 0:128],
                                                 in1=mask)
                        pends.append((w, qoff, kb, eg[:, j, :]))
                    if g == 0 and deferred[0] is not None:
                        flush_and_finalize(*deferred[0])
                        deferred[0] = None
                    while len(pends) > 4:
                        pw, poff, pkb, pe = pends.pop(0)
                        nc.tensor.matmul(
                            get_o_ps(o_box)[:, poff:poff + pw],
                            v_sb[:, pkb, 0:HD + 1],
                            pe[:, 0:pw],
                            start=(pkb == 0), stop=False)
                    pull_steps({0: 8, 1: 5, 2: 3, 3: 2}[st])
                deferred[0] = (b, st, v_sb, o_box, pends)
            flush_and_finalize(*deferred[0], last=True)

    nc.compile()
    return nc


def _to_bf16(a):
    return np.asarray(a, dtype=np.float32).astype(bf16_np)


def _pack_consts(wq, wk, wv_, bq, bk, bv):
    cbt = np.zeros((128, CB), dtype=np.float32)
    # wk[p, db, :] = Wk[db*128+p, :]
    cbt[:, 0:NDB * 64] = (
        wk.reshape(NDB, 128, HD).transpose(1, 0, 2).reshape(128, NDB * 64))
    cbt[:, 512:640] = np.eye(128, dtype=np.float32)
    # wqv[p, db, 0:64] = Wq[db*128+p, :]; [64:128] = Wv
    wqv = np.concatenate(
        [wq.reshape(NDB, 128, HD).transpose(1, 0, 2),
         wv_.reshape(NDB, 128, HD).transpose(1, 0, 2)], axis=2)
    cbt[:, 640:640 + NDB * 128] = wqv.reshape(128, NDB * 128)
    # mask[k, q] = 1.0 where q >= k (keep)
    cbt[:, 1664:1792] = (np.arange(128)[None, :] >= np.arange(128)[:, None])
    cbt[:, 1792:1792 + NKB] = 1.0

    cft = np.zeros((128, 2), dtype=np.float32)
    cft[0:HD, 0] = bk
    cft[:, 1] = np.concatenate([bq, bv])
    return cbt.astype(bf16_np), np.ascontiguousarray(cft)


def kernel(x, Wq, bq, Wk, bk, Wv, bv):
    if "nc" not in _cache:
        _cache["nc"] = _build()
    nc = _cache["nc"]

    cbt, cft = _pack_consts(
        np.asarray(Wq, np.float32), np.asarray(Wk, np.float32),
        np.asarray(Wv, np.float32), np.asarray(bq, np.float32),
        np.asarray(bk, np.float32), np.asarray(bv, np.float32))

    # X^T layout per core: [128 p, b, db, s] with p the within-block d index
    xb = _to_bf16(x)  # [B, S, D]
    in_maps = []
    for c in range(N_CORES):
        xc = xb[c * BPC:(c + 1) * BPC]  # [BPC, S, D]
        # [128 p, b, st, db, s_local]
        xt = np.ascontiguousarray(
            xc.reshape(BPC, NST, ST, NDB, 128).transpose(4, 0, 1, 3, 2))
        in_maps.append({"xT": xt, "cb": cbt, "cf": cft})

    res = bass_utils.run_bass_kernel_spmd(nc, in_maps,
                                          core_ids=list(range(N_CORES)),
                                          **_cache.get("run_kwargs", {}))
    _cache["last_result"] = res
    return np.concatenate([res.results[c]["out"] for c in range(N_CORES)],
                          axis=0)
